# revision 1
# baseline (speedup 1.0000x reference)
"""Trainium2 Bass kernel for nn_CNNInteractLayer (CNN interaction layer).

Math: for each episode b, s-row i, q-row j:
  out[b,i,j] = maxpool_L(relu(conv_k(concat(s[b,i], q[b,j])))) for k in 2..5
Key factorization: conv(concat(s,q)) = conv_s(s) + conv_q(q) + bias, so we
compute per-row convolutions once (25+13 rows per core instead of 625 pairs)
and form pairwise sums with a 0/1 selection matmul on the PE.

All operands are bf16 (PE runs bf16 at the same 1 cycle/row as fp32r but
DMA traffic halves); PSUM accumulation stays fp32. The max over the L=31
window drains PSUM through Act (copy to bf16 SBUF + deferred DVE max
tree, scheme A) and DVE (direct tensor_reduce, scheme R) in a balanced
mix; relu runs on the host. The real backend allows at most one PSUM
operand per vector op and no gpsimd PSUM access, which rules out
two-operand PSUM folds and gpsimd offload.

Sharding: 8 cores = 4 episodes x 2 halves of the q-row range.
"""

import os
import sys

import numpy as np

for _p in ("/opt/trn_rl_repo",):
    if os.path.isdir(_p) and _p not in sys.path:
        sys.path.insert(0, _p)

# the bass runner needs the axon jax backend; don't let a cpu-only pin hide it
if "axon" not in os.environ.get("JAX_PLATFORMS", "axon"):
    os.environ.pop("JAX_PLATFORMS", None)

import ml_dtypes  # noqa: E402

from concourse import bacc, bass, mybir, tile  # noqa: E402
from concourse.bass_utils import run_bass_kernel_spmd  # noqa: E402

BF16 = ml_dtypes.bfloat16

# Problem dims (hardcoded per spec)
B, N, K, Q, L, D = 4, 5, 5, 5, 31, 512
NROW = N * K            # 25 s-rows per episode
NQROW = N * Q           # 25 q-rows per episode
JN = 13                 # q-rows per core (padded; odd cores use 12)
ROWSTR = L + 2          # padded row stride (2-wide zero gap shared L/R)
SLAB_S = NROW * ROWSTR + 2   # 827 conv output positions (incl. row gaps)
SLAB_Q = JN * ROWSTR + 2     # 431
PS_COLS = 832           # input cols: data at 33r+2..33r+32, +/-2 halo
PQ_COLS = 436
NCH = 600               # device channels: [k5 | k4 | k3 | k2] x 150
# delta (tap shift) groups; prefix-size in device channel order
DELTAS = [(-2, 300), (-1, 600), (0, 600), (1, 450), (2, 150)]
# emission order per d-chunk: full-coverage groups first so the first matmul
# of each PSUM accumulation group writes the full partition range
DORDER = [1, 2, 0, 3, 4]
WOFF = [0, 300, 900, 1500, 1950]  # packed col offset of each delta group
WSIDE = 2100
CC0 = [0, 128, 256, 384, 512]     # channel chunk starts
CCW = [128, 128, 128, 128, 88]
XROWS = 39                        # 25 s + 13 q + 1 bias
NPAIR = NROW * JN                 # 325
# uniform 110-row chunks (last padded: 325 real pairs, 330 dma'd rows) so
# the whole output flushes in one DMA from a single [110, 3, 600] red tile
MCH = [(0, 110), (110, 110), (220, 105)]
PAD_OF_K = {2: 1, 3: 1, 4: 2, 5: 2}
ORD_OF_K = {5: 0, 4: 1, 3: 2, 2: 3}
POSCH_S = [(0, 512), (512, 315)]
POSCH_Q = [(0, 431)]
SUBW = 496                        # pairwise n-subchunk: 16 channel groups
CORDER = [3, 2, 1, 0, 4]          # conv/pairwise chunk processing order
WARMN = 14                        # PE warmup matmuls (p-state ramp filler)
# (mi, cc) -> reduce scheme. A: Act copies the 31 psum cols to sbuf bf16
# and a deferred DVE tree reduces them. R: direct DVE tensor_reduce from
# psum. ~2/3 A balances Act and DVE; consecutive groups interleave over mi
# so the two drain engines alternate and the psum pool recycles at PE
# pace. gpsimd supports neither PSUM access nor TensorTensor on the real
# backend, so it can't help with the reduce.
SCHEME_TABLE = {
    (0, 0): "R", (1, 0): "A", (2, 0): "R",
    (0, 1): "A", (1, 1): "R", (2, 1): "R",
    (0, 2): "R", (1, 2): "A", (2, 2): "A",
    (0, 3): "A", (1, 3): "R", (2, 3): "A",
    (0, 4): "A", (1, 4): "R", (2, 4): "A",
}

# chunk-major packed-W layout: per channel chunk, [side s | side q], each a
# concatenation of the valid delta groups' column slices for that chunk
def _chunk_tables():
    chw = []          # per-side width of each chunk block
    coloff = {}       # (cc, side, di) -> column offset in packed W
    off = 0
    for cc in range(5):
        c0 = CC0[cc]
        widths = []
        for di, (_, sz) in enumerate(DELTAS):
            w = min(128, sz - c0) if sz > c0 else 0
            widths.append(w)
        side_w = sum(widths)
        for side in range(2):
            p = off + side * side_w
            for di, w in enumerate(widths):
                if w:
                    coloff[(cc, side, di)] = p
                    p += w
        chw.append(side_w)
        off += 2 * side_w
    return chw, coloff


CHW, WCOL = _chunk_tables()
CHOFF = [sum(2 * w for w in CHW[:i]) for i in range(6)]

_PROG = None


def _sub_plan(cc):
    """(offset, width) n-subchunks within an X chunk + psum bank grouping."""
    ccw = CCW[cc]
    total = ccw * 31
    subs = []
    off = 0
    while off < total:
        w = min(SUBW, total - off)
        subs.append((off, w))
        off += w
    # pairs of equal-width subchunks share a 2-bank psum tile: the two
    # matmuls stay within their own banks (bank-crossing matmuls are
    # illegal) while the drain covers both halves in one instruction
    groups = []
    i = 0
    while i < len(subs):
        g = [i]
        if i + 1 < len(subs) and subs[i + 1][1] == subs[i][1]:
            g.append(i + 1)
        groups.append(g)
        i += len(g)
    return subs, groups


def _build_program():
    nc = bacc.Bacc("TRN2", target_bir_lowering=False, debug=False, num_devices=8)
    f32 = mybir.dt.float32
    bf16 = mybir.dt.bfloat16

    ps_d = nc.dram_tensor("ps", [D, PS_COLS], bf16, kind="ExternalInput")
    pq_d = nc.dram_tensor("pq", [D, PQ_COLS], bf16, kind="ExternalInput")
    w_d = nc.dram_tensor("w", [D, 2 * WSIDE], bf16, kind="ExternalInput")
    a_d = nc.dram_tensor("a", [XROWS, NPAIR], bf16, kind="ExternalInput")
    bias_d = nc.dram_tensor("bias", [1, 5 * 128 * 31], bf16, kind="ExternalInput")
    x_dram = nc.dram_tensor("xstage", [XROWS - 1, 5 * 128 * 31], bf16)
    out_d = nc.dram_tensor("out", [330, NCH], bf16, kind="ExternalOutput")

    with tile.TileContext(nc) as tc:
        with (
            tc.tile_pool(name="persist", bufs=1) as big,
            tc.tile_pool(name="xpool", bufs=3) as xpool,
            tc.tile_pool(name="sapool", bufs=4) as sapool,
            tc.tile_pool(name="redpool", bufs=1) as redpool,
            tc.tile_pool(name="convps", bufs=2, space="PSUM") as convps,
            tc.tile_pool(name="pwps", bufs=3, space="PSUM") as pwps,
        ):
            w_sb = big.tile([128, 4 * 2 * WSIDE], bf16, tag="w")
            ps_sb = big.tile([128, 4 * PS_COLS], bf16, tag="ps")
            pq_sb = big.tile([128, 4 * PQ_COLS], bf16, tag="pq")
            cs_sb = big.tile([128, 5 * SLAB_S], bf16, tag="cs")
            cq_sb = big.tile([128, 5 * SLAB_Q], bf16, tag="cq")
            a_sb = big.tile([XROWS, NPAIR], bf16, tag="a")

            # keep the PE busy during the input-DMA prologue so the clock
            # ramp (0.65/1.2 -> 2.4 GHz after 3us busy) is spent on filler
            # and the first conv matmul lands at full speed
            warm_sb = big.tile([128, 256], bf16, tag="warm")
            warm_ps = convps.tile([128, 512], f32, tag="conv")
            nc.gpsimd.memset(warm_sb[:], 0.0)
            for _wi in range(WARMN):
                nc.tensor.matmul(
                    warm_ps[0:128, 0:256],
                    lhsT=warm_sb[:, 0:128],
                    rhs=warm_sb[:, 0:256],
                    start=True,
                    stop=True,
                )

            def wload(cc):
                wd = w_d[:].rearrange("(d p) c -> p d c", p=128)
                ws = w_sb[:].rearrange("p (d c) -> p d c", c=2 * WSIDE)
                # per-d slices: each d-group of conv matmuls only waits on
                # its own quarter of the chunk's weights
                for d in range(4):
                    nc.sync.dma_start(
                        ws[:, d : d + 1, CHOFF[cc] : CHOFF[cc + 1]],
                        wd[:, d : d + 1, CHOFF[cc] : CHOFF[cc + 1]],
                    )

            # conv chunk order: mid-size weight chunks first so the first
            # conv starts early; cc4 (smallest pairwise) last for a short
            # tail after the final flush split. The q side (smaller input)
            # loads and runs first, and the first chunk's weights plus the
            # s input stream in per-d slices so the first conv matmuls (d0)
            # start as soon as ~1/4 of their data has landed.
            nc.sync.dma_start(
                pq_sb[:].rearrange("p (d c) -> p d c", c=PQ_COLS),
                pq_d[:].rearrange("(d p) c -> p d c", p=128),
            )
            wload(CORDER[0])
            ps_r = ps_d[:].rearrange("(d p) c -> p d c", p=128)
            pss_r = ps_sb[:].rearrange("p (d c) -> p d c", c=PS_COLS)
            for d in range(4):
                nc.sync.dma_start(
                    pss_r[:, d : d + 1, :], ps_r[:, d : d + 1, :]
                )
            nc.sync.dma_start(a_sb[:], a_d[:])
            wload(CORDER[1])

            def conv_chunk(cc, side, pci):
                """One conv pos-chunk (matmuls + Act copy) for (cc, side)."""
                c0, ccw = CC0[cc], CCW[cc]
                src, dst, poschunks, cols, slab = (
                    (ps_sb, cs_sb, POSCH_S, PS_COLS, SLAB_S)
                    if side == 0
                    else (pq_sb, cq_sb, POSCH_Q, PQ_COLS, SLAB_Q)
                )
                pos0, pw = poschunks[pci]
                psum = convps.tile([128, 512], f32, tag="conv")
                mms = []
                for d in range(4):
                    # first and last matmul of the accumulation group must
                    # cover the full partition range (start/stop semantics
                    # are per-element), so full-size delta groups bracket
                    order = DORDER if d < 3 else [1, 0, 3, 4, 2]
                    for di in order:
                        delta, sz = DELTAS[di]
                        if sz <= c0:
                            continue
                        wcc = min(ccw, sz - c0)
                        mms.append((d, di, delta, wcc))
                for idx, (d, di, delta, wcc) in enumerate(mms):
                    lcol = d * 2 * WSIDE + WCOL[(cc, side, di)]
                    rcol = d * cols + pos0 + delta + 2
                    nc.tensor.matmul(
                        psum[0:wcc, 0:pw],
                        lhsT=w_sb[:, lcol : lcol + wcc],
                        rhs=src[:, rcol : rcol + pw],
                        start=(idx == 0),
                        stop=(idx == len(mms) - 1),
                    )
                nc.scalar.copy(
                    dst[0:ccw, cc * slab + pos0 : cc * slab + pos0 + pw],
                    psum[0:ccw, 0:pw],
                )

            def xevict(cc, side):
                """conv_sb -> DRAM staging in X[row, slot*31 + l] layout.

                One DMA per side, issued as soon as that side's conv copies
                land: DRAM write APs have no partition-dim ordering
                constraint, so (p, r, l) iteration can scatter to row-major
                X. Keeps total DMA count (per-DMA HWDGE fixed cost) low.
                """
                xc0 = cc * 128 * 31
                ccw = CCW[cc]
                if side == 0:
                    nc.sync.dma_start(
                        bass.AP(
                            x_dram[:].tensor,
                            xc0,
                            [[31, ccw], [5 * 128 * 31, NROW], [1, 31]],
                        ),
                        bass.AP(
                            cs_sb[:].tensor,
                            cs_sb[:].offset + cc * SLAB_S,
                            [[cs_sb[:].ap[0][0], ccw], [ROWSTR, NROW], [1, 31]],
                        ),
                    )
                else:
                    nc.sync.dma_start(
                        bass.AP(
                            x_dram[:].tensor,
                            NROW * 5 * 128 * 31 + xc0,
                            [[31, ccw], [5 * 128 * 31, JN], [1, 31]],
                        ),
                        bass.AP(
                            cq_sb[:].tensor,
                            cq_sb[:].offset + cc * SLAB_Q,
                            [[cq_sb[:].ap[0][0], ccw], [ROWSTR, JN], [1, 31]],
                        ),
                    )

            def xload(cc):
                """DRAM staging -> X tile [39, 3968] (contiguous rows)."""
                xc0 = cc * 128 * 31
                w = CCW[cc] * 31
                xt = xpool.tile([XROWS, 128 * 31], bf16, tag="x")
                # column slices: early pairwise units only wait on the
                # leading slice of the X tile
                bounds = [0]
                step = (w // 3 + 991) // 992 * 992
                while bounds[-1] + step < w:
                    bounds.append(bounds[-1] + step)
                bounds.append(w)
                for b0, b1 in zip(bounds[:-1], bounds[1:]):
                    nc.sync.dma_start(
                        xt[0 : XROWS - 1, b0:b1],
                        x_dram[:, xc0 + b0 : xc0 + b1],
                    )
                nc.sync.dma_start(
                    xt[XROWS - 1 : XROWS, 0:w],
                    bias_d[0:1, xc0 : xc0 + w],
                )
                return xt

            def tree(sa, c0l, ccw, d0, msz, red_out, eng):
                """bf16 max tree over the innermost dim: d0 -> 1, covering
                sa columns [c0l, c0l+ccw).

                bf16 SBUF tensor_tensor gets the DVE 2x perf mode; overlap
                of the two halves at odd widths is harmless for max.
                """
                cur, d, cb = sa, d0, c0l
                while d > 1:
                    w = (d + 1) // 2
                    if w == 1:
                        out = red_out
                        eng.tensor_tensor(
                            out,
                            cur[0:msz, cb : cb + ccw, 0:1],
                            cur[0:msz, cb : cb + ccw, d - 1 : d],
                            mybir.AluOpType.max,
                        )
                    else:
                        out = sapool.tile([110, ccw, w], bf16, tag=f"t{w}")
                        eng.tensor_tensor(
                            out[0:msz, 0:ccw, 0:w],
                            cur[0:msz, cb : cb + ccw, 0:w],
                            cur[0:msz, cb : cb + ccw, d - w : d],
                            mybir.AluOpType.max,
                        )
                    cur, d, cb = out, w, 0

            def pw_units(cc, xt):
                """Pairwise emission units + deferred tree thunk for cc.

                Per (mi, group): matmul into psum, then drain. The first
                NA_TABLE[cc] groups use scheme A (Act copies the 31 psum
                cols to sbuf bf16, a deferred DVE tree reduces them); the
                rest use scheme R (direct DVE tensor_reduce). A/R groups
                alternate Act and DVE as psum-drainers so the pool recycles
                at PE pace; the R-suffix leaves a tree-free tail.
                """
                subs, groups = _sub_plan(cc)
                ccw = CCW[cc]
                sas = {
                    mi: sapool.tile(
                        [110, ccw, 31], bf16, tag=f"sa{mi}", name=f"sa{mi}_{cc}"
                    )
                    for mi in range(3)
                    if SCHEME_TABLE[(mi, cc)] in ("A", "C")
                }
                units = []
                for gi, g in enumerate(groups):
                    for mi, (moff, msz) in enumerate(MCH):
                        def u(gi=gi, g=g, mi=mi, moff=moff, msz=msz):
                            ng = len(g)
                            gsw = subs[g[0]][1]
                            nchs = gsw // 31
                            cl = subs[g[0]][0] // 31
                            pw = pwps.tile([110, 2, 512], f32, tag="pw")
                            for j, si in enumerate(g):
                                nc.tensor.matmul(
                                    pw[0:msz, j : j + 1, 0:gsw],
                                    lhsT=a_sb[:, moff : moff + msz],
                                    rhs=xt[
                                        :, subs[si][0] : subs[si][0] + gsw
                                    ],
                                    start=True,
                                    stop=True,
                                )
                            base = pw[0:msz, 0:ng, 0:gsw]
                            pv = bass.AP(
                                base.tensor,
                                base.offset,
                                [
                                    [base.ap[0][0], msz],
                                    [512, ng],
                                    [31, nchs],
                                    [1, 31],
                                ],
                            )
                            sch = SCHEME_TABLE[(mi, cc)]
                            if sch == "A":
                                nc.scalar.copy(
                                    sas[mi][
                                        0:msz, cl : cl + ng * nchs, 0:31
                                    ],
                                    pv,
                                )
                            elif sch == "C":
                                # fold 31->16 with one psum operand per
                                # instr: Act copies psum[0:16], DVE maxes
                                # psum[15:31] in place (l=15 overlap is
                                # harmless for max)
                                pv16a = bass.AP(
                                    base.tensor, base.offset,
                                    [[base.ap[0][0], msz], [512, ng],
                                     [31, nchs], [1, 16]],
                                )
                                pv16b = bass.AP(
                                    base.tensor, base.offset + 15,
                                    [[base.ap[0][0], msz], [512, ng],
                                     [31, nchs], [1, 16]],
                                )
                                sv = sas[mi][
                                    0:msz, cl : cl + ng * nchs, 0:16
                                ]
                                nc.scalar.copy(sv, pv16a)
                                nc.vector.tensor_tensor(
                                    sv, pv16b, sv, mybir.AluOpType.max
                                )
                            else:
                                cb = CC0[cc] + cl
                                nc.vector.tensor_reduce(
                                    red_all[
                                        0:msz, mi, cb : cb + ng * nchs
                                    ],
                                    pv,
                                    axis=mybir.AxisListType.X,
                                    op=mybir.AluOpType.max,
                                )

                        units.append(u)

                def treepart(c0l, w):
                    for mi, (moff, msz) in enumerate(MCH):
                        sch = SCHEME_TABLE[(mi, cc)]
                        if sch == "R":
                            continue
                        c0 = CC0[cc] + c0l
                        tree(
                            sas[mi], c0l, w, 31 if sch == "A" else 16, msz,
                            red_all[0:msz, mi, c0 : c0 + w],
                            nc.vector,
                        )

                def trees():
                    treepart(0, ccw)

                return units, trees, treepart

            red_all = redpool.tile([110, 3, NCH], bf16, tag="red")
            reds = red_all
            # rows 105:110 of the last pair chunk are padding the flush DMA
            # ships anyway; initialize them once
            nc.gpsimd.memset(red_all[96:110, 2, :], 0.0)

            # software-pipelined emission: conv leads xbuild by 1 chunk,
            # pairwise lags conv by 2 chunks (keeps PE fed while X DMAs land)
            def flush(c0, c1):
                """One output DMA for finished red columns (relu on host)."""
                w = c1 - c0
                pstr = red_all[:].ap[0][0]
                nc.sync.dma_start(
                    bass.AP(
                        out_d[:].tensor,
                        c0,
                        [[NCH, 110], [110 * NCH, 3], [1, w]],
                    ),
                    bass.AP(
                        red_all[:].tensor,
                        red_all[:].offset + c0,
                        [[pstr, 110], [NCH, 3], [1, w]],
                    ),
                )

            xts = {}
            cc0_ = CORDER[0]
            conv_chunk(cc0_, 1, 0)
            xevict(cc0_, 1)
            conv_chunk(cc0_, 0, 0)
            conv_chunk(cc0_, 0, 1)
            xevict(cc0_, 0)
            xts[cc0_] = xload(cc0_)
            pending = None
            for i in range(1, 5):
                cc = CORDER[i]
                if i + 1 <= 4:
                    wload(CORDER[i + 1])
                prev = CORDER[i - 1]
                units, trees, _tp = pw_units(prev, xts.pop(prev))
                n = len(units)
                a, b = n // 3, (2 * n) // 3
                if i == 4:
                    # last conv is short and prev's X lands late: emit all
                    # conv chunks first so the units never wait on the load
                    conv_chunk(cc, 0, 0)
                    conv_chunk(cc, 0, 1)
                    xevict(cc, 0)
                    conv_chunk(cc, 1, 0)
                    xevict(cc, 1)
                    xts[cc] = xload(cc)
                    for u in units:
                        u()
                else:
                    conv_chunk(cc, 0, 0)
                    for u in units[:a]:
                        u()
                    conv_chunk(cc, 0, 1)
                    for u in units[a:b]:
                        u()
                    xevict(cc, 0)
                    conv_chunk(cc, 1, 0)
                    for u in units[b:]:
                        u()
                    xevict(cc, 1)
                    xts[cc] = xload(cc)
                if pending is not None:
                    pending()
                pending = trees
            units, trees, treepart = pw_units(CORDER[4], xts.pop(CORDER[4]))
            pending()  # last deferred tree overlaps the final chunk's mms
            for u in units:
                u()
            flush(0, 512)
            trees()
            flush(512, NCH)

    nc.compile()
    return nc


def get_program():
    global _PROG
    if _PROG is None:
        _PROG = _build_program()
    return _PROG


def build_inputs(s, q, ws, bs):
    """Host-side shard prep. ws/bs: dicts k -> w(150, 1024, k) / b(150,).

    Returns in_maps. Core c handles episode c//2, q-row half c%2.
    """
    s = np.asarray(s, dtype=np.float32).reshape(B, NROW, L, D)
    q = np.asarray(q, dtype=np.float32).reshape(B, NQROW, L, D)

    # packed weights [D, 2*2100]: per side, delta groups at WOFF offsets,
    # device channel order [k5|k4|k3|k2]
    wall = np.zeros((D, 2 * WSIDE), dtype=np.float32)
    bias_dev = np.zeros(NCH, dtype=np.float32)
    for k in (2, 3, 4, 5):
        blk = ORD_OF_K[k] * 150
        bias_dev[blk : blk + 150] = bs[k]
        for di, (delta, sz) in enumerate(DELTAS):
            t = delta + PAD_OF_K[k]
            if not (0 <= t < k):
                continue
            assert blk + 150 <= sz
            wall[:, WOFF[di] + blk : WOFF[di] + blk + 150] = ws[k][:, :D, t].T
            wall[:, WSIDE + WOFF[di] + blk : WSIDE + WOFF[di] + blk + 150] = (
                ws[k][:, D:, t].T
            )
    perm = np.zeros(2 * WSIDE, dtype=np.int64)
    for side in range(2):
        for di, (_, sz) in enumerate(DELTAS):
            for cc in range(5):
                c0 = CC0[cc]
                if sz <= c0:
                    continue
                w = min(128, sz - c0)
                newc = WCOL[(cc, side, di)]
                oldc = side * WSIDE + WOFF[di] + c0
                perm[newc : newc + w] = np.arange(oldc, oldc + w)
    wall = wall[:, perm].astype(BF16)

    bias_pad = np.zeros(5 * 128 * 31, dtype=np.float32)
    bias_pad[: NCH * 31] = np.repeat(bias_dev, 31)
    bias_rep = bias_pad[None, :].astype(BF16)

    amat = np.zeros((XROWS, NPAIR), dtype=np.float32)
    for i in range(NROW):
        for t in range(JN):
            p = i * JN + t
            amat[i, p] = 1.0
            amat[NROW + t, p] = 1.0
    amat[XROWS - 1, :] = 1.0
    amat = amat.astype(BF16)

    in_maps = []
    for core in range(8):
        b, jh = core // 2, core % 2
        jidx = [min(jh * JN + t, NQROW - 1) for t in range(JN)]
        psa = np.zeros((D, PS_COLS), dtype=np.float32)
        pqa = np.zeros((D, PQ_COLS), dtype=np.float32)
        for r in range(NROW):
            psa[:, r * ROWSTR + 2 : r * ROWSTR + 2 + L] = s[b, r].T
        for t, j in enumerate(jidx):
            pqa[:, t * ROWSTR + 2 : t * ROWSTR + 2 + L] = q[b, j].T
        in_maps.append(
            {
                "ps": psa.astype(BF16),
                "pq": pqa.astype(BF16),
                "w": wall,
                "a": amat,
                "bias": bias_rep,
            }
        )
    return in_maps


# device channel -> original output channel maps
_S_IDX = np.array(
    [(3 - g) * 150 + u for g in range(4) for u in range(75)], dtype=np.int64
)
_Q_IDX = _S_IDX + 75


def assemble_outputs(core_outs):
    """core_outs: list of 8 arrays [NPAIR, NCH] -> (s_out, q_out)."""
    s_out = np.empty((B, NROW, NQROW, 300), dtype=np.float32)
    q_out = np.empty((B, NROW, NQROW, 300), dtype=np.float32)
    for core in range(8):
        b, jh = core // 2, core % 2
        nj = JN if jh == 0 else NQROW - JN
        arr = np.ascontiguousarray(core_outs[core])[:NPAIR].astype(np.float32)
        np.maximum(arr, 0.0, out=arr)  # relu (device ships pre-relu maxes)
        arr = arr.reshape(NROW, JN, NCH)
        s_out[b, :, jh * JN : jh * JN + nj] = arr[:, :nj][:, :, _S_IDX]
        q_out[b, :, jh * JN : jh * JN + nj] = arr[:, :nj][:, :, _Q_IDX]
    return s_out.reshape(-1, 300), q_out.reshape(-1, 300)


def kernel(s, q, w2, b2, w3, b3, w4, b4, w5, b5, B=4, N=5, K=5, Q=5, L=31):
    ws = {2: np.asarray(w2, np.float32), 3: np.asarray(w3, np.float32),
          4: np.asarray(w4, np.float32), 5: np.asarray(w5, np.float32)}
    bs = {2: np.asarray(b2, np.float32), 3: np.asarray(b3, np.float32),
          4: np.asarray(b4, np.float32), 5: np.asarray(b5, np.float32)}
    in_maps = build_inputs(s, q, ws, bs)
    nc = get_program()
    res = run_bass_kernel_spmd(nc, in_maps, list(range(8))).results
    return assemble_outputs([res[c]["out"] for c in range(8)])

